# revision 29
# baseline (speedup 1.0000x reference)
"""Trainium2 Bass kernel for CorrectedPartialCharges.

out[i] = pc[i] + (total_charge[g] - seg_sum[g]) / n_atoms[g],  g = i // 256

Sharding: graphs are data-parallel across the 8 cores (4096 graphs /
1,048,576 atoms per core); segment sums and the gather-broadcast stay
device-local. On each core, partition p owns 32 contiguous graphs.

Wire format is bf16 (the 2e-2 rel-err budget allows it): node charges are
rounded to bf16 on the host, halving HBM traffic both ways; all device
accumulation is fp32. total_charge is pre-divided by 256 on the host so
the leftover is one fused scalar_tensor_tensor op.

Schedule model: the kernel is HBM-bound (4 MiB/core at ~360 GB/s of DMA no
matter how loads/stores interleave), so the goals are (a) keep the SDMA
streams saturated end-to-end and (b) keep the post-last-load tail short.
  - One HWDGE ring saturates at ~200 GB/s, so the payload is striped over
    BOTH rings (sync + ACT). Per ring, all loads are queued first and
    stores queue behind them: loads run at full rate, then the store
    backlog drains immediately - no idle gap.
  - The identity for the matmul fold is built on-chip (memset +
    affine_select): a DMA'd identity is 128 tiny descriptors that
    round-robin against the ring's 4KB packets and halve its bandwidth.
    The small total-charge vector rides the sync ring right after the
    first tile.
  - A dummy ACTIVATE is emitted right after the load issues so the ~1.3us
    ACT_TABLE_LOAD happens while the first tiles are still in flight
    instead of lazily right before the first real bias-add.
  - Tensor: accumulating identity matmuls fold each graph's 256 atoms into
    PSUM_W PSUM columns. Vector: PSUM reduce -> seg, fused leftover, and
    the per-graph adds for the two wide tiles. ACT: per-graph bias adds
    for the three narrower tiles (its adds are ~2x slower per block, so
    it never owns a 2048-wide tile). GpSimd only builds the identity
    (its tensor_scalar is ~4us/block and numerically broken here).
  - Adds write a fresh tile instead of in-place: in-place adds + split
    stores produced intermittent wrong results (observed rel-err 0.07 on
    one run, correct on re-run), and the one-writer-per-tile dependency
    graph avoids that hazard class entirely.
  - The last tile is small and its store is striped across both rings to
    shorten the drain tail; run-to-run exec variance is ~+-1.5us from
    hardware DMA-utilization throttling (duty-cycled to 50% for ~35% of
    the kernel in every observed trace), so the schedule is chosen for
    the best 5-run mean (~27.8us), not a lucky single run.
"""

import ml_dtypes
import numpy as np

import concourse.bacc as bacc
import concourse.bass as bass  # noqa: F401
import concourse.masks as masks
import concourse.mybir as mybir
import concourse.tile as tile
from concourse.bass_utils import run_bass_kernel_spmd

N_CORES = 8
ATOMS_PER_GRAPH = 256
N_GRAPHS = 32768
N_ATOMS = N_GRAPHS * ATOMS_PER_GRAPH
P = 128

G_PER_CORE = N_GRAPHS // N_CORES          # 4096 graphs per core
A_PER_CORE = G_PER_CORE * ATOMS_PER_GRAPH  # 1,048,576 atoms per core

# Knobs read by test.py when experimenting.
# Tile widths by index (atoms per partition); tiles are contiguous spans of
# the free dim in index order. Sum must be 8192; each a multiple of 256 with
# width/256 * PSUM_W * 4 <= 2048 (PSUM accumulation bank limit).
TILE_W = (1536, 2048, 2048, 1536, 1024)
PSUM_W = 32               # columns per graph after the matmul pre-reduce
# processing (emission) order of the tiles
ORDER = (0, 1, 3, 2, 4)
# add engine per tile ("vector" | "scalar" | "gpsimd")
TILE_ADD_ENGINE = {0: "scalar", 1: "vector", 2: "vector", 3: "scalar", 4: "scalar"}
# HWDGE ring per tile for load / store ("sync" | "scalar")
LOAD_RING = {0: "sync", 1: "scalar", 2: "scalar", 3: "sync", 4: "sync"}
STORE_RING = {0: "sync", 1: "scalar", 2: "scalar", 3: "sync", 4: "sync"}
TC_RING = "sync"          # ring for the total-charge vector (after 1st load)
WARMUP_MM = 0             # dummy matmuls to spin the PE clock up pre-load
SPLIT_LAST_STORE = True   # stripe the last tile's store across both rings
INPLACE_ADDS = False      # in-place adds cap DVE tensor_scalar at 2x mode

_TRACE = False
_TRACE_KWARGS = {}


def _build(tile_w=None, order=None, add_eng=None, psum_w=None, load_ring=None,
           store_ring=None, tc_ring=None, warmup_mm=None, split_last=None,
           inplace=None):
    tile_w = TILE_W if tile_w is None else tile_w
    order = ORDER if order is None else order
    add_eng = TILE_ADD_ENGINE if add_eng is None else add_eng
    psum_w = PSUM_W if psum_w is None else psum_w
    load_ring = LOAD_RING if load_ring is None else load_ring
    store_ring = STORE_RING if store_ring is None else store_ring
    tc_ring = TC_RING if tc_ring is None else tc_ring
    warmup_mm = WARMUP_MM if warmup_mm is None else warmup_mm
    split_last = SPLIT_LAST_STORE if split_last is None else split_last
    inplace = INPLACE_ADDS if inplace is None else inplace

    nt = len(tile_w)
    ap_free = A_PER_CORE // P     # 8192 atoms per partition
    gp = G_PER_CORE // P          # 32 graphs per partition
    n_pass = ATOMS_PER_GRAPH // psum_w
    offs = [0]
    for w_ in tile_w:
        offs.append(offs[-1] + w_)
    assert offs[-1] == ap_free
    for w_ in tile_w:
        assert w_ % ATOMS_PER_GRAPH == 0
        assert (w_ // ATOMS_PER_GRAPH) * psum_w * 4 <= 2048, \
            "psum accumulation group must fit one bank"
    assert tuple(sorted(order)) == tuple(range(nt))

    nc = bacc.Bacc(None, target_bir_lowering=False, enable_partition_id=False)

    pc = nc.dram_tensor("pc", [A_PER_CORE], mybir.dt.bfloat16, kind="ExternalInput")
    # total_charge / 256, fp32
    tcs = nc.dram_tensor("tcs", [G_PER_CORE], mybir.dt.float32, kind="ExternalInput")
    out = nc.dram_tensor("out", [A_PER_CORE], mybir.dt.bfloat16, kind="ExternalOutput")

    pc_v = pc[:].rearrange("(p n) -> p n", p=P)
    out_v = out[:].rearrange("(p n) -> p n", p=P)
    tcs_v = tcs[:].rearrange("(p k) -> p k", p=P)

    with tile.TileContext(nc) as tc:
        with (
            tc.tile_pool(name="io", bufs=nt) as io_pool,
            tc.tile_pool(name="small", bufs=2 * nt) as small_pool,
            tc.tile_pool(name="consts", bufs=1) as const_pool,
            tc.tile_pool(name="psum", bufs=min(nt, 6), space="PSUM") as psum_pool,
        ):
            # Identity built on-chip on the (idle at t=0) GpSimd engine.
            eye_tile = const_pool.tile([P, P], mybir.dt.bfloat16, tag="eye")
            masks.make_identity(nc, eye_tile[:])

            # Dummy matmuls while the loads are in flight: the PE clock
            # ramps with activity (HAM), so without these the first tiles'
            # folds run at half pitch.
            if warmup_mm:
                with tc.tile_pool(name="wps", bufs=1, space="PSUM") as wpool:
                    wps = wpool.tile([P, P], mybir.dt.float32, tag="wps")
                    for _ in range(warmup_mm):
                        nc.tensor.matmul(
                            wps[:], eye_tile[:], eye_tile[:], start=True,
                            stop=True,
                        )

            # Queue every input tile load up front, striped over both HWDGE
            # rings; stores are issued later on the same rings and queue
            # behind the loads. The total-charge vector rides tc_ring right
            # after that ring's first tile.
            xs = []
            tc_tile = const_pool.tile([P, gp], mybir.dt.float32, tag="tc")
            tc_loaded = False
            if tc_ring == "gpsimd":
                nc.gpsimd.dma_start(out=tc_tile[:], in_=tcs_v)
                tc_loaded = True
            for t in range(nt):
                w_ = tile_w[t]
                x = io_pool.tile([P, w_], mybir.dt.bfloat16, tag="x")
                getattr(nc, load_ring[t]).dma_start(
                    out=x[:], in_=pc_v[:, offs[t] : offs[t] + w_]
                )
                xs.append(x)
                if not tc_loaded and load_ring[t] == tc_ring:
                    getattr(nc, tc_ring).dma_start(out=tc_tile[:], in_=tcs_v)
                    tc_loaded = True

            # Dummy ACTIVATE: hoists the ~1.3us ACT table load to overlap
            # the in-flight tile loads instead of gating the first real add.
            dummy = const_pool.tile([P, 1], mybir.dt.bfloat16, tag="dummy")
            nc.scalar.add(out=dummy[:], in_=eye_tile[:, 0:1], add=0.0)

            goff = [o // ATOMS_PER_GRAPH for o in offs]  # graph offsets
            for t in order:
                x = xs[t]
                w_ = tile_w[t]
                k = w_ // ATOMS_PER_GRAPH
                x3 = x[:].rearrange("p (k a) -> p k a", a=ATOMS_PER_GRAPH)

                # Fold 256 atoms -> psum_w columns per graph with
                # accumulating identity matmuls on the Tensor engine.
                ps = psum_pool.tile([P, k, psum_w], mybir.dt.float32, tag="ps")
                for s in range(n_pass):
                    nc.tensor.matmul(
                        ps[:],
                        eye_tile[:],
                        x3[:, :, s * psum_w : (s + 1) * psum_w],
                        start=(s == 0),
                        stop=(s == n_pass - 1),
                    )
                seg = small_pool.tile([P, k], mybir.dt.float32, tag="seg")
                nc.vector.reduce_sum(
                    out=seg[:], in_=ps[:], axis=mybir.AxisListType.X
                )

                # left = (seg * -1/256) + tc/256   (fused)
                left = small_pool.tile([P, k], mybir.dt.float32, tag="left")
                nc.vector.scalar_tensor_tensor(
                    out=left[:],
                    in0=seg[:],
                    scalar=-1.0 / ATOMS_PER_GRAPH,
                    in1=tc_tile[:, goff[t] : goff[t] + k],
                    op0=mybir.AluOpType.mult,
                    op1=mybir.AluOpType.add,
                )

                if inplace:
                    y = x
                else:
                    y = io_pool.tile([P, w_], mybir.dt.bfloat16, tag="y")
                for j in range(k):
                    blk = x[:, j * ATOMS_PER_GRAPH : (j + 1) * ATOMS_PER_GRAPH]
                    oblk = y[:, j * ATOMS_PER_GRAPH : (j + 1) * ATOMS_PER_GRAPH]
                    if add_eng[t] == "scalar":
                        nc.scalar.add(out=oblk, in_=blk, add=left[:, j : j + 1])
                    elif add_eng[t] == "gpsimd":
                        nc.gpsimd.tensor_scalar_add(
                            out=oblk, in0=blk, scalar1=left[:, j : j + 1]
                        )
                    else:
                        nc.vector.tensor_scalar_add(
                            out=oblk, in0=blk, scalar1=left[:, j : j + 1]
                        )
                if split_last and t == order[-1]:
                    hw_ = w_ // 2
                    nc.sync.dma_start(
                        out=out_v[:, offs[t] : offs[t] + hw_], in_=y[:, 0:hw_]
                    )
                    nc.scalar.dma_start(
                        out=out_v[:, offs[t] + hw_ : offs[t] + w_],
                        in_=y[:, hw_:w_],
                    )
                else:
                    getattr(nc, store_ring[t]).dma_start(
                        out=out_v[:, offs[t] : offs[t] + w_], in_=y[:]
                    )

    nc.finalize()
    return nc


_NC_CACHE = {}


def _get_nc():
    key = (TILE_W, ORDER, tuple(sorted(TILE_ADD_ENGINE.items())), PSUM_W,
           tuple(sorted(LOAD_RING.items())), tuple(sorted(STORE_RING.items())),
           TC_RING, WARMUP_MM, SPLIT_LAST_STORE, INPLACE_ADDS)
    if key not in _NC_CACHE:
        _NC_CACHE[key] = _build()
    return _NC_CACHE[key]


def _cpu_fallback(pc, total_charge, batch, n_atoms):
    num_segments = n_atoms.shape[0]
    seg = np.bincount(batch, weights=pc.astype(np.float64), minlength=num_segments)
    leftover = (total_charge - seg.astype(np.float32)) / n_atoms.astype(np.float32)
    return (pc + leftover[batch]).astype(np.float32)


def kernel(**inputs) -> np.ndarray:
    pc = np.ascontiguousarray(
        np.asarray(inputs["node_outputs"], dtype=np.float32).reshape(-1)
    )
    total_charge = np.ascontiguousarray(
        np.asarray(inputs["total_charge"], dtype=np.float32).reshape(-1)
    )
    batch = np.asarray(inputs["batch"]).reshape(-1)
    n_atoms = np.ascontiguousarray(np.asarray(inputs["n_atoms"], dtype=np.int32).reshape(-1))

    # The device kernel hardcodes the uniform 256-atoms-per-graph layout the
    # reference generator produces; anything else goes through numpy.
    if (
        pc.shape[0] != N_ATOMS
        or total_charge.shape[0] != N_GRAPHS
        or not np.all(n_atoms == ATOMS_PER_GRAPH)
        or not np.array_equal(
            batch.astype(np.int64),
            np.arange(N_ATOMS, dtype=np.int64) // ATOMS_PER_GRAPH,
        )
    ):
        return _cpu_fallback(pc, total_charge, batch, n_atoms)

    pc_b = pc.astype(ml_dtypes.bfloat16)
    tcs = (total_charge * (1.0 / ATOMS_PER_GRAPH)).astype(np.float32)

    nc = _get_nc()
    in_maps = []
    for c in range(N_CORES):
        in_maps.append(
            {
                "pc": pc_b[c * A_PER_CORE : (c + 1) * A_PER_CORE],
                "tcs": tcs[c * G_PER_CORE : (c + 1) * G_PER_CORE],
            }
        )
    res = run_bass_kernel_spmd(
        nc, in_maps, list(range(N_CORES)), trace=_TRACE, **_TRACE_KWARGS
    )
    out = np.concatenate([r["out"] for r in res.results]).astype(np.float32)
    if _TRACE:
        kernel.last_results = res
    return out


# revision 36
# speedup vs baseline: 1.1297x; 1.1297x over previous
"""Trainium2 Bass kernel for CorrectedPartialCharges.

out[i] = pc[i] + (total_charge[g] - seg_sum[g]) / n_atoms[g],  g = i // 256

Sharding: graphs are data-parallel across the 8 cores (4096 graphs /
1,048,576 atoms per core); segment sums and the gather-broadcast stay
device-local. On each core, partition p owns 32 contiguous graphs.

Wire format is bf16 (the 2e-2 rel-err budget allows it): node charges are
rounded to bf16 on the host, halving HBM traffic both ways; all device
accumulation is fp32. total_charge is pre-divided by 256 on the host so
the leftover is one fused scalar_tensor_tensor op.

Schedule model: the kernel is HBM-bound (4 MiB/core at ~360 GB/s of DMA no
matter how loads/stores interleave), so the goals are (a) keep the SDMA
streams saturated end-to-end and (b) keep the post-last-load tail short.
  - One HWDGE ring saturates at ~200 GB/s, so the payload is striped over
    BOTH rings (sync + ACT). Per ring, all loads are queued first and
    stores queue behind them: loads run at full rate, then the store
    backlog drains immediately - no idle gap.
  - The identity for the matmul fold is built on-chip (memset +
    affine_select) and the small total-charge vector rides the SWDGE
    queue: any 128-tiny-descriptor DMA placed on a HWDGE ring round-robins
    against that ring's 4KB packets at packet granularity and stalls it
    for ~2-3us mid-stream (measured).
  - A dummy ACTIVATE is emitted right after the load issues so the ~1.3us
    ACT_TABLE_LOAD happens while the first tiles are still in flight
    instead of lazily right before the first real bias-add.
  - Tensor: accumulating identity matmuls fold each graph's 256 atoms into
    PSUM_W PSUM columns. Vector: PSUM reduce -> seg, fused leftover, and
    the per-graph adds for the wide tiles + the last tile. ACT: per-graph
    bias adds for tiles 0 and 3 (its adds are ~2x slower per block, so it
    never owns a 2048-wide tile). Tile 3's reduce is emitted AHEAD of
    tile 1's add chain (the "h"/"t" phases in ORDER) so ACT's second
    chain is released ~1.5us earlier - worth ~0.5us end to end. GpSimd
    only builds the identity and loads the total-charge vector (its
    tensor_scalar is ~4us/block and numerically broken here).
  - Adds write a fresh tile instead of in-place: in-place adds + split
    stores produced intermittent wrong results (observed rel-err 0.07 on
    one run, correct on re-run), and the one-writer-per-tile dependency
    graph avoids that hazard class entirely.
  - The last tile is small and its store is striped across both rings to
    shorten the drain tail; run-to-run exec variance is ~+-1.5us from
    hardware DMA-utilization throttling (duty-cycled to 50% for ~35% of
    the kernel in every observed trace), so the schedule is chosen for
    the best 5-run mean (~27.8us), not a lucky single run.
"""

import ml_dtypes
import numpy as np

import concourse.bacc as bacc
import concourse.bass as bass  # noqa: F401
import concourse.masks as masks
import concourse.mybir as mybir
import concourse.tile as tile
from concourse.bass_utils import run_bass_kernel_spmd

N_CORES = 8
ATOMS_PER_GRAPH = 256
N_GRAPHS = 32768
N_ATOMS = N_GRAPHS * ATOMS_PER_GRAPH
P = 128

G_PER_CORE = N_GRAPHS // N_CORES          # 4096 graphs per core
A_PER_CORE = G_PER_CORE * ATOMS_PER_GRAPH  # 1,048,576 atoms per core

# Knobs read by test.py when experimenting.
# Tile widths by index (atoms per partition); tiles are contiguous spans of
# the free dim in index order. Sum must be 8192; each a multiple of 256 with
# width/256 * PSUM_W * 4 <= 2048 (PSUM accumulation bank limit).
TILE_W = (1024, 2048, 2048, 1792, 1280)
PSUM_W = 32               # columns per graph after the matmul pre-reduce
# processing (emission) order of the tiles; each entry is (tile, part) with
# part "h" (fold+reduce+leftover) or "t" (adds+store), letting a later
# tile's reduce be hoisted ahead of an earlier tile's add chain so the ACT
# engine's chain is released sooner.
ORDER = ((0, "h"), (0, "t"), (1, "h"), (3, "h"), (3, "t"), (1, "t"),
         (2, "h"), (2, "t"), (4, "h"), (4, "t"))
# add engine per tile ("vector" | "scalar" | "gpsimd")
TILE_ADD_ENGINE = {0: "scalar", 1: "vector", 2: "vector", 3: "scalar", 4: "vector"}
# HWDGE ring per tile for load / store ("sync" | "scalar")
LOAD_RING = {0: "sync", 1: "scalar", 2: "scalar", 3: "sync", 4: "sync"}
STORE_RING = {0: "sync", 1: "scalar", 2: "scalar", 3: "sync", 4: "sync"}
TC_RING = "gpsimd"        # SWDGE: keeps 128 tiny descriptors off the rings
WARMUP_MM = 0             # dummy matmuls to spin the PE clock up pre-load
SPLIT_LAST_STORE = True   # stripe the last tile's store across both rings
INPLACE_ADDS = False      # in-place adds cap DVE tensor_scalar at 2x mode

_TRACE = False
_TRACE_KWARGS = {}


def _build(tile_w=None, order=None, add_eng=None, psum_w=None, load_ring=None,
           store_ring=None, tc_ring=None, warmup_mm=None, split_last=None,
           inplace=None):
    tile_w = TILE_W if tile_w is None else tile_w
    order = ORDER if order is None else order
    add_eng = TILE_ADD_ENGINE if add_eng is None else add_eng
    psum_w = PSUM_W if psum_w is None else psum_w
    load_ring = LOAD_RING if load_ring is None else load_ring
    store_ring = STORE_RING if store_ring is None else store_ring
    tc_ring = TC_RING if tc_ring is None else tc_ring
    warmup_mm = WARMUP_MM if warmup_mm is None else warmup_mm
    split_last = SPLIT_LAST_STORE if split_last is None else split_last
    inplace = INPLACE_ADDS if inplace is None else inplace

    nt = len(tile_w)
    ap_free = A_PER_CORE // P     # 8192 atoms per partition
    gp = G_PER_CORE // P          # 32 graphs per partition
    n_pass = ATOMS_PER_GRAPH // psum_w
    offs = [0]
    for w_ in tile_w:
        offs.append(offs[-1] + w_)
    assert offs[-1] == ap_free
    for w_ in tile_w:
        assert w_ % ATOMS_PER_GRAPH == 0
        assert (w_ // ATOMS_PER_GRAPH) * psum_w * 4 <= 2048, \
            "psum accumulation group must fit one bank"
    assert sorted(t for t, p in order if p == "h") == list(range(nt))
    assert sorted(t for t, p in order if p == "t") == list(range(nt))
    seen_h = set()
    for t, p in order:
        if p == "h":
            seen_h.add(t)
        else:
            assert t in seen_h, f"tile {t} adds emitted before its reduce"

    nc = bacc.Bacc(None, target_bir_lowering=False, enable_partition_id=False)

    pc = nc.dram_tensor("pc", [A_PER_CORE], mybir.dt.bfloat16, kind="ExternalInput")
    # total_charge / 256, fp32
    tcs = nc.dram_tensor("tcs", [G_PER_CORE], mybir.dt.float32, kind="ExternalInput")
    out = nc.dram_tensor("out", [A_PER_CORE], mybir.dt.bfloat16, kind="ExternalOutput")

    pc_v = pc[:].rearrange("(p n) -> p n", p=P)
    out_v = out[:].rearrange("(p n) -> p n", p=P)
    tcs_v = tcs[:].rearrange("(p k) -> p k", p=P)

    with tile.TileContext(nc) as tc:
        with (
            tc.tile_pool(name="io", bufs=nt) as io_pool,
            tc.tile_pool(name="small", bufs=2 * nt) as small_pool,
            tc.tile_pool(name="consts", bufs=1) as const_pool,
            tc.tile_pool(name="psum", bufs=min(nt, 6), space="PSUM") as psum_pool,
        ):
            # Identity built on-chip on the (idle at t=0) GpSimd engine.
            eye_tile = const_pool.tile([P, P], mybir.dt.bfloat16, tag="eye")
            masks.make_identity(nc, eye_tile[:])

            # Dummy matmuls while the loads are in flight: the PE clock
            # ramps with activity (HAM), so without these the first tiles'
            # folds run at half pitch.
            if warmup_mm:
                with tc.tile_pool(name="wps", bufs=1, space="PSUM") as wpool:
                    wps = wpool.tile([P, P], mybir.dt.float32, tag="wps")
                    for _ in range(warmup_mm):
                        nc.tensor.matmul(
                            wps[:], eye_tile[:], eye_tile[:], start=True,
                            stop=True,
                        )

            # Queue every input tile load up front, striped over both HWDGE
            # rings; stores are issued later on the same rings and queue
            # behind the loads. The total-charge vector rides tc_ring right
            # after that ring's first tile.
            xs = []
            tc_tile = const_pool.tile([P, gp], mybir.dt.float32, tag="tc")
            tc_loaded = False
            if tc_ring == "gpsimd":
                nc.gpsimd.dma_start(out=tc_tile[:], in_=tcs_v)
                tc_loaded = True
            for t in range(nt):
                w_ = tile_w[t]
                x = io_pool.tile([P, w_], mybir.dt.bfloat16, tag="x")
                getattr(nc, load_ring[t]).dma_start(
                    out=x[:], in_=pc_v[:, offs[t] : offs[t] + w_]
                )
                xs.append(x)
                if not tc_loaded and load_ring[t] == tc_ring:
                    getattr(nc, tc_ring).dma_start(out=tc_tile[:], in_=tcs_v)
                    tc_loaded = True

            # Dummy ACTIVATE: hoists the ~1.3us ACT table load to overlap
            # the in-flight tile loads instead of gating the first real add.
            dummy = const_pool.tile([P, 1], mybir.dt.bfloat16, tag="dummy")
            nc.scalar.add(out=dummy[:], in_=eye_tile[:, 0:1], add=0.0)

            goff = [o // ATOMS_PER_GRAPH for o in offs]  # graph offsets
            lefts = {}
            last_t = [t for t, p in order if p == "t"][-1]
            for t, part in order:
                x = xs[t]
                w_ = tile_w[t]
                k = w_ // ATOMS_PER_GRAPH
                if part == "h":
                    x3 = x[:].rearrange("p (k a) -> p k a", a=ATOMS_PER_GRAPH)
                    # Fold 256 atoms -> psum_w columns per graph with
                    # accumulating identity matmuls on the Tensor engine.
                    ps = psum_pool.tile([P, k, psum_w], mybir.dt.float32,
                                        tag="ps")
                    for s in range(n_pass):
                        nc.tensor.matmul(
                            ps[:],
                            eye_tile[:],
                            x3[:, :, s * psum_w : (s + 1) * psum_w],
                            start=(s == 0),
                            stop=(s == n_pass - 1),
                        )
                    seg = small_pool.tile([P, k], mybir.dt.float32, tag="seg")
                    nc.vector.reduce_sum(
                        out=seg[:], in_=ps[:], axis=mybir.AxisListType.X
                    )
                    # left = (seg * -1/256) + tc/256   (fused)
                    left = small_pool.tile([P, k], mybir.dt.float32, tag="left")
                    nc.vector.scalar_tensor_tensor(
                        out=left[:],
                        in0=seg[:],
                        scalar=-1.0 / ATOMS_PER_GRAPH,
                        in1=tc_tile[:, goff[t] : goff[t] + k],
                        op0=mybir.AluOpType.mult,
                        op1=mybir.AluOpType.add,
                    )
                    lefts[t] = left
                    continue

                left = lefts[t]
                if inplace:
                    y = x
                else:
                    y = io_pool.tile([P, w_], mybir.dt.bfloat16, tag="y")
                for j in range(k):
                    blk = x[:, j * ATOMS_PER_GRAPH : (j + 1) * ATOMS_PER_GRAPH]
                    oblk = y[:, j * ATOMS_PER_GRAPH : (j + 1) * ATOMS_PER_GRAPH]
                    if add_eng[t] == "scalar":
                        nc.scalar.add(out=oblk, in_=blk, add=left[:, j : j + 1])
                    elif add_eng[t] == "gpsimd":
                        nc.gpsimd.tensor_scalar_add(
                            out=oblk, in0=blk, scalar1=left[:, j : j + 1]
                        )
                    else:
                        nc.vector.tensor_scalar_add(
                            out=oblk, in0=blk, scalar1=left[:, j : j + 1]
                        )
                if split_last and t == last_t:
                    hw_ = w_ // 2
                    nc.sync.dma_start(
                        out=out_v[:, offs[t] : offs[t] + hw_], in_=y[:, 0:hw_]
                    )
                    nc.scalar.dma_start(
                        out=out_v[:, offs[t] + hw_ : offs[t] + w_],
                        in_=y[:, hw_:w_],
                    )
                else:
                    getattr(nc, store_ring[t]).dma_start(
                        out=out_v[:, offs[t] : offs[t] + w_], in_=y[:]
                    )

    nc.finalize()
    return nc


_NC_CACHE = {}


def _get_nc():
    key = (TILE_W, ORDER, tuple(sorted(TILE_ADD_ENGINE.items())), PSUM_W,
           tuple(sorted(LOAD_RING.items())), tuple(sorted(STORE_RING.items())),
           TC_RING, WARMUP_MM, SPLIT_LAST_STORE, INPLACE_ADDS)
    if key not in _NC_CACHE:
        _NC_CACHE[key] = _build()
    return _NC_CACHE[key]


def _cpu_fallback(pc, total_charge, batch, n_atoms):
    num_segments = n_atoms.shape[0]
    seg = np.bincount(batch, weights=pc.astype(np.float64), minlength=num_segments)
    leftover = (total_charge - seg.astype(np.float32)) / n_atoms.astype(np.float32)
    return (pc + leftover[batch]).astype(np.float32)


def kernel(**inputs) -> np.ndarray:
    pc = np.ascontiguousarray(
        np.asarray(inputs["node_outputs"], dtype=np.float32).reshape(-1)
    )
    total_charge = np.ascontiguousarray(
        np.asarray(inputs["total_charge"], dtype=np.float32).reshape(-1)
    )
    batch = np.asarray(inputs["batch"]).reshape(-1)
    n_atoms = np.ascontiguousarray(np.asarray(inputs["n_atoms"], dtype=np.int32).reshape(-1))

    # The device kernel hardcodes the uniform 256-atoms-per-graph layout the
    # reference generator produces; anything else goes through numpy.
    if (
        pc.shape[0] != N_ATOMS
        or total_charge.shape[0] != N_GRAPHS
        or not np.all(n_atoms == ATOMS_PER_GRAPH)
        or not np.array_equal(
            batch.astype(np.int64),
            np.arange(N_ATOMS, dtype=np.int64) // ATOMS_PER_GRAPH,
        )
    ):
        return _cpu_fallback(pc, total_charge, batch, n_atoms)

    pc_b = pc.astype(ml_dtypes.bfloat16)
    tcs = (total_charge * (1.0 / ATOMS_PER_GRAPH)).astype(np.float32)

    nc = _get_nc()
    in_maps = []
    for c in range(N_CORES):
        in_maps.append(
            {
                "pc": pc_b[c * A_PER_CORE : (c + 1) * A_PER_CORE],
                "tcs": tcs[c * G_PER_CORE : (c + 1) * G_PER_CORE],
            }
        )
    res = run_bass_kernel_spmd(
        nc, in_maps, list(range(N_CORES)), trace=_TRACE, **_TRACE_KWARGS
    )
    out = np.concatenate([r["out"] for r in res.results]).astype(np.float32)
    if _TRACE:
        kernel.last_results = res
    return out


# revision 42
# speedup vs baseline: 1.1728x; 1.0382x over previous
"""Trainium2 Bass kernel for CorrectedPartialCharges.

out[i] = pc[i] + (total_charge[g] - seg_sum[g]) / n_atoms[g],  g = i // 256

Sharding: graphs are data-parallel across the 8 cores (4096 graphs /
1,048,576 atoms per core); segment sums and the gather-broadcast stay
device-local. On each core, partition p owns 32 contiguous graphs.

Wire format is bf16 (the 2e-2 rel-err budget allows it): node charges are
rounded to bf16 on the host, halving HBM traffic both ways; all device
accumulation is fp32. total_charge is pre-divided by 256 on the host so
the leftover is one fused scalar_tensor_tensor op.

Schedule model: the kernel is HBM-bound (4 MiB/core at ~360 GB/s of DMA no
matter how loads/stores interleave), so the goals are (a) keep the SDMA
streams saturated end-to-end and (b) keep the post-last-load tail short.
  - One HWDGE ring saturates at ~200 GB/s, so the payload is striped over
    BOTH rings (sync + ACT). Per ring, all loads are queued first and
    stores queue behind them: loads run at full rate, then the store
    backlog drains immediately - no idle gap.
  - The identity for the matmul fold is built on-chip (memset +
    affine_select) and the small total-charge vector rides the SWDGE
    queue: any 128-tiny-descriptor DMA placed on a HWDGE ring round-robins
    against that ring's 4KB packets at packet granularity and stalls it
    for ~2-3us mid-stream (measured).
  - A dummy ACTIVATE is emitted right after the load issues so the ~1.3us
    ACT_TABLE_LOAD happens while the first tiles are still in flight
    instead of lazily right before the first real bias-add.
  - Tensor: accumulating identity matmuls fold each graph's 256 atoms into
    PSUM_W PSUM columns. Vector: PSUM reduce -> seg, fused leftover, and
    the per-graph adds for the wide tiles + the last tile. ACT: per-graph
    bias adds for tiles 0 and 3 (its adds are ~2x slower per block, so it
    never owns a 2048-wide tile). Tile 3's reduce is emitted AHEAD of
    tile 1's add chain (the "h"/"t" phases in ORDER) so ACT's second
    chain is released ~1.5us earlier - worth ~0.5us end to end. GpSimd
    only builds the identity and loads the total-charge vector (its
    tensor_scalar is ~4us/block and numerically broken here).
  - Adds write a fresh tile instead of in-place: in-place adds + split
    stores produced intermittent wrong results (observed rel-err 0.07 on
    one run, correct on re-run), and the one-writer-per-tile dependency
    graph avoids that hazard class entirely.
  - The last tile is small and its store is striped across both rings to
    shorten the drain tail; run-to-run exec variance is ~+-1.5us from
    hardware DMA-utilization throttling (duty-cycled to 50% for ~35% of
    the kernel in every observed trace), so the schedule is chosen for
    the best 5-run mean (~27.8us), not a lucky single run.
"""

import ml_dtypes
import numpy as np

import concourse.bacc as bacc
import concourse.bass as bass  # noqa: F401
import concourse.masks as masks
import concourse.mybir as mybir
import concourse.tile as tile
from concourse.bass_utils import run_bass_kernel_spmd

N_CORES = 8
ATOMS_PER_GRAPH = 256
N_GRAPHS = 32768
N_ATOMS = N_GRAPHS * ATOMS_PER_GRAPH
P = 128

G_PER_CORE = N_GRAPHS // N_CORES          # 4096 graphs per core
A_PER_CORE = G_PER_CORE * ATOMS_PER_GRAPH  # 1,048,576 atoms per core

# Knobs read by test.py when experimenting.
# Tile widths by index (atoms per partition); tiles are contiguous spans of
# the free dim in index order. Sum must be 8192; each a multiple of 256 with
# width/256 * PSUM_W * 4 <= 2048 (PSUM accumulation bank limit).
TILE_W = (1024, 2048, 2048, 1792, 1280)
PSUM_W = 32               # columns per graph after the matmul pre-reduce
# processing (emission) order of the tiles; each entry is (tile, part) with
# part "h" (fold+reduce+leftover) or "t" (adds+store), letting a later
# tile's reduce be hoisted ahead of an earlier tile's add chain so the ACT
# engine's chain is released sooner.
ORDER = ((0, "h"), (0, "t"), (1, "h"), (3, "h"), (3, "t"), (1, "t"),
         (2, "h"), (2, "t"), (4, "h"), (4, "t"))
# add engine per tile ("vector" | "scalar" | "gpsimd")
TILE_ADD_ENGINE = {0: "scalar", 1: "vector", 2: "vector", 3: "scalar", 4: "vector"}
# HWDGE ring per tile for load / store ("sync" | "scalar")
LOAD_RING = {0: "sync", 1: "scalar", 2: "scalar", 3: "sync", 4: "sync"}
STORE_RING = {0: "sync", 1: "scalar", 2: "scalar", 3: "sync", 4: "sync"}
TC_RING = "gpsimd"        # SWDGE: keeps 128 tiny descriptors off the rings
WARMUP_MM = 0             # dummy matmuls to spin the PE clock up pre-load
SPLIT_LAST_STORE = True   # stripe the last tile's store across both rings
INPLACE_ADDS = False      # in-place adds cap DVE tensor_scalar at 2x mode

_TRACE = False
_TRACE_KWARGS = {}


def _build(tile_w=None, order=None, add_eng=None, psum_w=None, load_ring=None,
           store_ring=None, tc_ring=None, warmup_mm=None, split_last=None,
           inplace=None):
    tile_w = TILE_W if tile_w is None else tile_w
    order = ORDER if order is None else order
    add_eng = TILE_ADD_ENGINE if add_eng is None else add_eng
    psum_w = PSUM_W if psum_w is None else psum_w
    load_ring = LOAD_RING if load_ring is None else load_ring
    store_ring = STORE_RING if store_ring is None else store_ring
    tc_ring = TC_RING if tc_ring is None else tc_ring
    warmup_mm = WARMUP_MM if warmup_mm is None else warmup_mm
    split_last = SPLIT_LAST_STORE if split_last is None else split_last
    inplace = INPLACE_ADDS if inplace is None else inplace

    nt = len(tile_w)
    ap_free = A_PER_CORE // P     # 8192 atoms per partition
    gp = G_PER_CORE // P          # 32 graphs per partition
    n_pass = ATOMS_PER_GRAPH // psum_w
    offs = [0]
    for w_ in tile_w:
        offs.append(offs[-1] + w_)
    assert offs[-1] == ap_free
    for w_ in tile_w:
        assert w_ % ATOMS_PER_GRAPH == 0
        assert (w_ // ATOMS_PER_GRAPH) * psum_w * 4 <= 2048, \
            "psum accumulation group must fit one bank"
    assert sorted(t for t, p in order if p == "h") == list(range(nt))
    assert sorted(t for t, p in order if p == "t") == list(range(nt))
    seen_h = set()
    for t, p in order:
        if p == "h":
            seen_h.add(t)
        else:
            assert t in seen_h, f"tile {t} adds emitted before its reduce"

    nc = bacc.Bacc(None, target_bir_lowering=False, enable_partition_id=False)

    pc = nc.dram_tensor("pc", [A_PER_CORE], mybir.dt.bfloat16, kind="ExternalInput")
    # total_charge / 256, fp32
    tcs = nc.dram_tensor("tcs", [G_PER_CORE], mybir.dt.float32, kind="ExternalInput")
    out = nc.dram_tensor("out", [A_PER_CORE], mybir.dt.bfloat16, kind="ExternalOutput")

    pc_v = pc[:].rearrange("(p n) -> p n", p=P)
    out_v = out[:].rearrange("(p n) -> p n", p=P)
    tcs_v = tcs[:].rearrange("(p k) -> p k", p=P)

    with tile.TileContext(nc) as tc:
        with (
            tc.tile_pool(name="io", bufs=nt) as io_pool,
            tc.tile_pool(name="small", bufs=2 * nt) as small_pool,
            tc.tile_pool(name="consts", bufs=1) as const_pool,
            tc.tile_pool(name="psum", bufs=min(nt, 6), space="PSUM") as psum_pool,
        ):
            # Identity built on-chip on the (idle at t=0) GpSimd engine.
            eye_tile = const_pool.tile([P, P], mybir.dt.bfloat16, tag="eye")
            masks.make_identity(nc, eye_tile[:])

            # Dummy matmuls while the loads are in flight: the PE clock
            # ramps with activity (HAM), so without these the first tiles'
            # folds run at half pitch.
            if warmup_mm:
                with tc.tile_pool(name="wps", bufs=1, space="PSUM") as wpool:
                    wps = wpool.tile([P, P], mybir.dt.float32, tag="wps")
                    for _ in range(warmup_mm):
                        nc.tensor.matmul(
                            wps[:], eye_tile[:], eye_tile[:], start=True,
                            stop=True,
                        )

            # Queue every input tile load up front, striped over both HWDGE
            # rings; stores are issued later on the same rings and queue
            # behind the loads. The total-charge vector rides tc_ring right
            # after that ring's first tile.
            xs = []
            tc_tile = const_pool.tile([P, gp], mybir.dt.float32, tag="tc")
            tc_loaded = False
            if tc_ring == "gpsimd":
                nc.gpsimd.dma_start(out=tc_tile[:], in_=tcs_v)
                tc_loaded = True
            for t in range(nt):
                w_ = tile_w[t]
                x = io_pool.tile([P, w_], mybir.dt.bfloat16, tag="x")
                getattr(nc, load_ring[t]).dma_start(
                    out=x[:], in_=pc_v[:, offs[t] : offs[t] + w_]
                )
                xs.append(x)
                if not tc_loaded and load_ring[t] == tc_ring:
                    getattr(nc, tc_ring).dma_start(out=tc_tile[:], in_=tcs_v)
                    tc_loaded = True

            # Dummy ACTIVATE: hoists the ~1.3us ACT table load to overlap
            # the in-flight tile loads instead of gating the first real add.
            dummy = const_pool.tile([P, 1], mybir.dt.bfloat16, tag="dummy")
            nc.scalar.add(out=dummy[:], in_=eye_tile[:, 0:1], add=0.0)

            goff = [o // ATOMS_PER_GRAPH for o in offs]  # graph offsets
            lefts = {}
            last_t = [t for t, p in order if p == "t"][-1]
            for t, part in order:
                x = xs[t]
                w_ = tile_w[t]
                k = w_ // ATOMS_PER_GRAPH
                if part == "h":
                    x3 = x[:].rearrange("p (k a) -> p k a", a=ATOMS_PER_GRAPH)
                    # Fold 256 atoms -> psum_w columns per graph with
                    # accumulating identity matmuls on the Tensor engine.
                    ps = psum_pool.tile([P, k, psum_w], mybir.dt.float32,
                                        tag="ps")
                    for s in range(n_pass):
                        nc.tensor.matmul(
                            ps[:],
                            eye_tile[:],
                            x3[:, :, s * psum_w : (s + 1) * psum_w],
                            start=(s == 0),
                            stop=(s == n_pass - 1),
                        )
                    seg = small_pool.tile([P, k], mybir.dt.float32, tag="seg")
                    nc.vector.reduce_sum(
                        out=seg[:], in_=ps[:], axis=mybir.AxisListType.X
                    )
                    # left = (seg * -1/256) + tc/256   (fused)
                    left = small_pool.tile([P, k], mybir.dt.float32, tag="left")
                    nc.vector.scalar_tensor_tensor(
                        out=left[:],
                        in0=seg[:],
                        scalar=-1.0 / ATOMS_PER_GRAPH,
                        in1=tc_tile[:, goff[t] : goff[t] + k],
                        op0=mybir.AluOpType.mult,
                        op1=mybir.AluOpType.add,
                    )
                    lefts[t] = left
                    continue

                left = lefts[t]
                if inplace:
                    y = x
                else:
                    y = io_pool.tile([P, w_], mybir.dt.bfloat16, tag="y")
                for j in range(k):
                    blk = x[:, j * ATOMS_PER_GRAPH : (j + 1) * ATOMS_PER_GRAPH]
                    oblk = y[:, j * ATOMS_PER_GRAPH : (j + 1) * ATOMS_PER_GRAPH]
                    if add_eng[t] == "scalar":
                        nc.scalar.add(out=oblk, in_=blk, add=left[:, j : j + 1])
                    elif add_eng[t] == "gpsimd":
                        nc.gpsimd.tensor_scalar_add(
                            out=oblk, in0=blk, scalar1=left[:, j : j + 1]
                        )
                    else:
                        nc.vector.tensor_scalar_add(
                            out=oblk, in0=blk, scalar1=left[:, j : j + 1]
                        )
                if split_last and t == last_t:
                    hw_ = w_ // 2
                    nc.sync.dma_start(
                        out=out_v[:, offs[t] : offs[t] + hw_], in_=y[:, 0:hw_]
                    )
                    nc.scalar.dma_start(
                        out=out_v[:, offs[t] + hw_ : offs[t] + w_],
                        in_=y[:, hw_:w_],
                    )
                else:
                    getattr(nc, store_ring[t]).dma_start(
                        out=out_v[:, offs[t] : offs[t] + w_], in_=y[:]
                    )

    nc.finalize()
    return nc


_NC_CACHE = {}


def _get_nc():
    key = (TILE_W, ORDER, tuple(sorted(TILE_ADD_ENGINE.items())), PSUM_W,
           tuple(sorted(LOAD_RING.items())), tuple(sorted(STORE_RING.items())),
           TC_RING, WARMUP_MM, SPLIT_LAST_STORE, INPLACE_ADDS)
    if key not in _NC_CACHE:
        _NC_CACHE[key] = _build()
    return _NC_CACHE[key]


def _cpu_fallback(pc, total_charge, batch, n_atoms):
    num_segments = n_atoms.shape[0]
    seg = np.bincount(batch, weights=pc.astype(np.float64), minlength=num_segments)
    leftover = (total_charge - seg.astype(np.float32)) / n_atoms.astype(np.float32)
    return (pc + leftover[batch]).astype(np.float32)


def kernel(**inputs) -> np.ndarray:
    pc = np.ascontiguousarray(
        np.asarray(inputs["node_outputs"], dtype=np.float32).reshape(-1)
    )
    total_charge = np.ascontiguousarray(
        np.asarray(inputs["total_charge"], dtype=np.float32).reshape(-1)
    )
    batch = np.asarray(inputs["batch"]).reshape(-1)
    n_atoms = np.ascontiguousarray(np.asarray(inputs["n_atoms"], dtype=np.int32).reshape(-1))

    # The device kernel hardcodes the uniform 256-atoms-per-graph layout the
    # reference generator produces; anything else goes through numpy.
    if (
        pc.shape[0] != N_ATOMS
        or total_charge.shape[0] != N_GRAPHS
        or not np.all(n_atoms == ATOMS_PER_GRAPH)
        or not np.array_equal(
            batch.astype(np.int64),
            np.arange(N_ATOMS, dtype=np.int64) // ATOMS_PER_GRAPH,
        )
    ):
        return _cpu_fallback(pc, total_charge, batch, n_atoms)

    pc_b = pc.astype(ml_dtypes.bfloat16)
    tcs = (total_charge * (1.0 / ATOMS_PER_GRAPH)).astype(np.float32)

    nc = _get_nc()
    in_maps = []
    for c in range(N_CORES):
        in_maps.append(
            {
                "pc": pc_b[c * A_PER_CORE : (c + 1) * A_PER_CORE],
                "tcs": tcs[c * G_PER_CORE : (c + 1) * G_PER_CORE],
            }
        )
    res = run_bass_kernel_spmd(
        nc, in_maps, list(range(N_CORES)), trace=_TRACE, **_TRACE_KWARGS
    )
    out = np.concatenate([r["out"] for r in res.results]).astype(np.float32)
    if _TRACE:
        kernel.last_results = res
    return out
